# revision 18
# baseline (speedup 1.0000x reference)
"""Trainium2 Bass kernel for ANI-style per-species MLP (MoE routing).

Reference computation (dense form):
    h  = rep @ W1[e] + b1[e]          (no activation on layer-1 output)
    h  = relu(h @ Wh[e] + bh[e])
    en = h @ W2[e] + b2[e]
    out[b] = sum over atoms of en[b, a, species[b, a]]

Strategy: MoE dispatch on the host — gather atoms by species, split each
species' atom list evenly across the 8 NeuronCores, pad each per-species
group to a static capacity.  Each core runs the same Bass graph (SPMD):
for each species, grouped GEMMs (3-layer MLP) over that species' atom
columns, hidden dim on SBUF partitions so per-partition bias/ReLU
epilogues fuse the PSUM->SBUF copy.  Compute is bf16 with fp32 PSUM
accumulation (rel err vs the fp32 reference ~4e-3).  Per-atom energies
are combined (scatter-add per molecule) on the host.

DMA plan (hard-won constraints): a HWDGE queue sustains ~150-200 GB/s on
~1KB descriptor runs; every dma_start costs ~650ns of issuing-engine
time; and Tile has 8 HW-DMA semaphore lanes, so the (k+8)-th DMA's issue
waits for the k-th DMA's completion.  Hence: few, large, contiguous
transfers (one weight blob per species, one bias blob, one x blob per
species — species 0 split per chunk for a fast start), split across the
two HWDGE queues (sync engine: species 0,2; scalar engine: species 1,3)
in consumption order.  x is stored chunk-major [3, n] per chunk so every
transfer and every matmul rhs slice is contiguous.
"""

import numpy as np
import ml_dtypes

from concourse import bacc
import concourse.mybir as mybir
from concourse import tile
from concourse.bass_utils import run_bass_kernel_spmd

B, A, D, E = 32, 1024, 384, 4
H1, H2 = 256, 192
NCORES = 8
DCH = D // 128          # 3 chunks of the descriptor dim
H1CH = H1 // 128        # 2 chunks of hidden-1
H2CH = 2                # hidden-2 padded 192 -> 256 = 2 chunks
MAX_N = 512             # moving free dim per matmul (one fp32 PSUM bank)
WT_W1, WT_WH, WT_W2 = 0, DCH * H1CH * 128, DCH * H1CH * 128 + H1CH * H2CH * 128
WT_COLS = WT_W2 + H2CH  # 1282: per-species weight blob columns
BF16 = ml_dtypes.bfloat16

_graph_cache = {}
_last_run = {}


def _chunks(c):
    """Split capacity c into matmul column tiles of at most MAX_N.

    Smallest tile first so species 0's first x DMA is small and lands
    early.
    """
    sizes = []
    rem = c
    while rem > 0:
        n = min(MAX_N, rem)
        sizes.append(n)
        rem -= n
    sizes.reverse()
    out = []
    n0 = 0
    for n in sizes:
        out.append((n0, n))
        n0 += n
    return out


def _build_graph(caps):
    """One Bass graph, shared by all 8 cores (SPMD)."""
    offs = np.concatenate([[0], np.cumsum(caps)])
    ctot = int(offs[-1])
    f32, bf = mybir.dt.float32, mybir.dt.bfloat16
    Act = mybir.ActivationFunctionType

    nc = bacc.Bacc()
    xt_d = nc.declare_dram_parameter("xt", [128, DCH * ctot], bf, isOutput=False)
    wt_d = nc.declare_dram_parameter("wt", [128, E, WT_COLS], bf, isOutput=False)
    bias_d = nc.declare_dram_parameter("bias", [128, E, 5, 1], f32, isOutput=False)
    out_d = nc.declare_dram_parameter("out", [1, ctot], f32, isOutput=True)

    chunk_lists = [_chunks(int(caps[e])) for e in range(E)]

    with tile.TileContext(nc) as tc:
        with (
            tc.tile_pool(name="wp", bufs=1) as wp,
            tc.tile_pool(name="xp", bufs=1) as xp,
            tc.tile_pool(name="hp", bufs=2) as hp,
            tc.tile_pool(name="op", bufs=1) as op,
            tc.tile_pool(name="pp2", bufs=2, space="PSUM") as pp2,
            tc.tile_pool(name="pp1", bufs=1, space="PSUM") as pp1,
        ):
            wt_s = [
                wp.tile([128, WT_COLS], bf, tag=f"wt_{e}", name=f"wt_{e}")
                for e in range(E)
            ]
            bias_s = wp.tile([128, E, 5, 1], f32, tag="bias")

            # x tiles: species 0 per-chunk (fast start), others one blob.
            # x0_ts[ci] is [128, DCH, n]; xs_ts[e] is [128, DCH*caps[e]]
            # chunk-major: chunk ci occupies cols [DCH*n0, DCH*n0 + DCH*n)
            # laid out [DCH, n].
            x0_ts = [
                xp.tile([128, DCH, n], bf, tag=f"x0_{ci}", name=f"x0_{ci}")
                for ci, (n0, n) in enumerate(chunk_lists[0])
            ]
            xs_ts = [None] + [
                xp.tile(
                    [128, DCH * int(caps[e])], bf, tag=f"x_{e}", name=f"x_{e}"
                )
                for e in range(1, E)
            ]

            # --- all input DMA issues up front, two queues, in order ---
            # sync queue: wt0, x0 chunks, bias, wt2, x2
            # scalar queue: wt1, x1, wt3, x3
            nc.sync.dma_start(out=wt_s[0][:], in_=wt_d[:, 0])
            nc.scalar.dma_start(out=wt_s[1][:], in_=wt_d[:, 1])
            g0 = int(offs[0])
            for ci, (n0, n) in enumerate(chunk_lists[0]):
                c0 = DCH * (g0 + n0)
                nc.sync.dma_start(
                    out=x0_ts[ci][:], in_=xt_d[:, c0 : c0 + DCH * n]
                )
            nc.scalar.dma_start(
                out=xs_ts[1][:],
                in_=xt_d[:, DCH * int(offs[1]) : DCH * int(offs[2])],
            )
            nc.sync.dma_start(out=bias_s[:], in_=bias_d[:])
            nc.sync.dma_start(out=wt_s[2][:], in_=wt_d[:, 2])
            nc.scalar.dma_start(out=wt_s[3][:], in_=wt_d[:, 3])
            nc.sync.dma_start(
                out=xs_ts[2][:],
                in_=xt_d[:, DCH * int(offs[2]) : DCH * int(offs[3])],
            )
            nc.scalar.dma_start(
                out=xs_ts[3][:],
                in_=xt_d[:, DCH * int(offs[3]) : DCH * int(offs[4])],
            )

            # --- compute ---
            for e in range(E):
                g0 = int(offs[e])
                en_t = op.tile([1, int(caps[e])], f32, tag=f"en_{e}", name=f"en_{e}")
                for ci, (n0, n) in enumerate(chunk_lists[e]):
                    if e == 0:
                        def rhs_x(d, _t=x0_ts[ci]):
                            return _t[:, d, :]
                    else:
                        def rhs_x(d, _t=xs_ts[e], _n0=n0, _n=n):
                            return _t[:, DCH * _n0 + d * _n : DCH * _n0 + (d + 1) * _n]
                    h1_t = hp.tile([128, H1CH, n], bf, tag="h1", name="h1_t")
                    h2_t = hp.tile([128, H2CH, n], bf, tag="h2", name="h2_t")

                    # layer 1: h1.T = W1.T @ x.T + b1 (no activation)
                    for h in range(H1CH):
                        ps = pp2.tile([128, n], f32, tag=f"ph1_{h}", name="ph1")
                        for d in range(DCH):
                            c0 = WT_W1 + d * (H1CH * 128) + h * 128
                            nc.tensor.matmul(
                                ps[:],
                                lhsT=wt_s[e][:, c0 : c0 + 128],
                                rhs=rhs_x(d),
                                start=(d == 0),
                                stop=(d == DCH - 1),
                            )
                        if h == 0:
                            nc.scalar.activation(
                                h1_t[:, h, :], ps[:], Act.Identity,
                                bias=bias_s[:, e, h, :],
                            )
                        else:
                            nc.vector.tensor_scalar_add(
                                h1_t[:, h, :], ps[:], bias_s[:, e, h, :]
                            )

                    # layer 2: h2.T = relu(Wh.T @ h1.T + bh)
                    for m in range(H2CH):
                        ps = pp1.tile([128, n], f32, tag=f"ph2_{m}", name="ph2")
                        for k in range(H1CH):
                            c0 = WT_WH + k * (H2CH * 128) + m * 128
                            nc.tensor.matmul(
                                ps[:],
                                lhsT=wt_s[e][:, c0 : c0 + 128],
                                rhs=h1_t[:, k, :],
                                start=(k == 0),
                                stop=(k == H1CH - 1),
                            )
                        if m == 0:
                            nc.scalar.activation(
                                h2_t[:, m, :], ps[:], Act.Relu,
                                bias=bias_s[:, e, H1CH + m, :],
                            )
                        else:
                            nc.vector.tensor_scalar(
                                h2_t[:, m, :], ps[:], bias_s[:, e, H1CH + m, :],
                                0.0, mybir.AluOpType.add, mybir.AluOpType.max,
                            )

                    # layer 3: en = W2.T @ h2.T + b2
                    ps_e = pp2.tile([1, n], f32, tag="pen", name="pen")
                    for k in range(H2CH):
                        nc.tensor.matmul(
                            ps_e[:],
                            lhsT=wt_s[e][:, WT_W2 + k : WT_W2 + k + 1],
                            rhs=h2_t[:, k, :],
                            start=(k == 0),
                            stop=(k == H2CH - 1),
                        )
                    nc.scalar.activation(
                        en_t[:, n0 : n0 + n], ps_e[:], Act.Identity,
                        bias=bias_s[0:1, e, 4, :],
                    )
                nc.scalar.dma_start(
                    out=out_d[:, g0 : g0 + int(caps[e])], in_=en_t[:]
                )
    return nc


def _pack_weights(W1, b1, Wh, bh, W2, b2):
    W1 = np.asarray(W1, np.float32)
    b1 = np.asarray(b1, np.float32)
    Wh = np.asarray(Wh, np.float32)
    bh = np.asarray(bh, np.float32)
    W2 = np.asarray(W2, np.float32)
    b2 = np.asarray(b2, np.float32)

    # weight blob [128, E, WT_COLS]: w1 | wh(padded) | w2(padded)
    w1p = W1.reshape(E, DCH, 128, H1CH, 128).transpose(2, 0, 1, 3, 4).reshape(
        128, E, WT_WH
    )
    whpad = np.zeros((E, H1, H2CH * 128), np.float32)
    whpad[:, :, :H2] = Wh
    whp = whpad.reshape(E, H1CH, 128, H2CH, 128).transpose(2, 0, 1, 3, 4).reshape(
        128, E, WT_W2 - WT_WH
    )
    w2pad = np.zeros((E, H2CH * 128), np.float32)
    w2pad[:, :H2] = W2
    w2p = w2pad.reshape(E, H2CH, 128).transpose(2, 0, 1)
    wt = np.concatenate([w1p, whp, w2p], axis=2).astype(BF16)

    # bias blob [128, E, 5, 1]: b1 (2 chunks) | bh (2 chunks) | b2 (row 0)
    bias = np.zeros((128, E, 5, 1), np.float32)
    bias[:, :, 0:H1CH, 0] = b1.reshape(E, H1CH, 128).transpose(2, 0, 1)
    bhpad = np.zeros((E, H2CH * 128), np.float32)
    bhpad[:, :H2] = bh
    bias[:, :, H1CH : H1CH + H2CH, 0] = bhpad.reshape(E, H2CH, 128).transpose(
        2, 0, 1
    )
    bias[0, :, 4, 0] = b2
    return {"wt": np.ascontiguousarray(wt), "bias": bias}


def kernel(representation, species, W1, b1, Wh, bh, W2, b2):
    rep = np.ascontiguousarray(np.asarray(representation, np.float32)).reshape(
        B * A, D
    )
    spec = np.asarray(species).reshape(B * A)

    # --- dispatch: per-species atom lists, split evenly across cores ---
    idx_ce = [[None] * E for _ in range(NCORES)]
    for e in range(E):
        ide = np.nonzero(spec == e)[0]
        for c, part in enumerate(np.array_split(ide, NCORES)):
            idx_ce[c][e] = part
    caps = tuple(
        max(128, int(-(-max(len(idx_ce[c][e]) for c in range(NCORES)) // 64) * 64))
        for e in range(E)
    )
    offs = np.concatenate([[0], np.cumsum(caps)])
    ctot = int(offs[-1])
    chunk_lists = [_chunks(int(caps[e])) for e in range(E)]

    wdict = _pack_weights(W1, b1, Wh, bh, W2, b2)
    rep_bf = rep.astype(BF16)

    in_maps = []
    for c in range(NCORES):
        xt = np.zeros((128, DCH * ctot), BF16)
        for e in range(E):
            ids = idx_ce[c][e]
            for n0, n in chunk_lists[e]:
                sub = ids[n0 : n0 + n]
                r = len(sub)
                if r == 0:
                    continue
                blk = rep_bf[sub].reshape(r, DCH, 128).transpose(2, 1, 0)
                c0 = DCH * (int(offs[e]) + n0)
                for d in range(DCH):
                    xt[:, c0 + d * n : c0 + d * n + r] = blk[:, d, :]
        in_maps.append({"xt": xt, **wdict})

    key = caps
    if key not in _graph_cache:
        nc = _build_graph(caps)
        nc.finalize()
        _graph_cache[key] = nc
    nc = _graph_cache[key]

    res = run_bass_kernel_spmd(nc, in_maps, core_ids=list(range(NCORES)))
    _last_run.update(nc=nc, in_maps=in_maps, caps=caps)

    # --- combine: scatter-add per-atom energies into per-molecule sums ---
    out = np.zeros(B, np.float64)
    for c in range(NCORES):
        en = np.asarray(res.results[c]["out"], np.float64)[0]
        for e in range(E):
            ids = idx_ce[c][e]
            n = len(ids)
            if n:
                out += np.bincount(
                    ids // A,
                    weights=en[int(offs[e]) : int(offs[e]) + n],
                    minlength=B,
                )
    return out.astype(np.float32)


# revision 20
# speedup vs baseline: 1.2528x; 1.2528x over previous
"""Trainium2 Bass kernel for ANI-style per-species MLP (MoE routing).

Reference computation (dense form):
    h  = rep @ W1[e] + b1[e]          (no activation on layer-1 output)
    h  = relu(h @ Wh[e] + bh[e])
    en = h @ W2[e] + b2[e]
    out[b] = sum over atoms of en[b, a, species[b, a]]

Strategy: MoE dispatch on the host — gather atoms by species, split each
species' atom list evenly across the 8 NeuronCores, pad each per-species
group to a static capacity.  Each core runs the same Bass graph (SPMD):
for each species, grouped GEMMs (3-layer MLP) over that species' atom
columns, hidden dim on SBUF partitions so per-partition bias/ReLU
epilogues fuse the PSUM->SBUF copy.  Compute is bf16 with fp32 PSUM
accumulation (rel err vs the fp32 reference ~4e-3).  Per-atom energies
are combined (scatter-add per molecule) on the host.

DMA plan (hard-won constraints): a HWDGE queue sustains ~150-200 GB/s on
~1KB descriptor runs; every dma_start costs ~650ns of issuing-engine
time; and Tile has 8 HW-DMA semaphore lanes, so the (k+8)-th DMA's issue
waits for the k-th DMA's completion.  Hence: few, large, contiguous
transfers (one weight blob per species, one bias blob, one x blob per
species — species 0 split per chunk for a fast start), split across the
two HWDGE queues (sync engine: species 0,2; scalar engine: species 1,3)
in consumption order.  x is stored chunk-major [3, n] per chunk so every
transfer and every matmul rhs slice is contiguous.
"""

import numpy as np
import ml_dtypes

from concourse import bacc
import concourse.mybir as mybir
from concourse import tile
from concourse.bass_utils import run_bass_kernel_spmd

B, A, D, E = 32, 1024, 384, 4
H1, H2 = 256, 192
NCORES = 8
DCH = D // 128          # 3 chunks of the descriptor dim
H1CH = H1 // 128        # 2 chunks of hidden-1
H2CH = 2                # hidden-2 padded 192 -> 256 = 2 chunks
MAX_N = 512             # moving free dim per matmul (one fp32 PSUM bank)
WT_W1, WT_WH, WT_W2 = 0, DCH * H1CH * 128, DCH * H1CH * 128 + H1CH * H2CH * 128
WT_COLS = WT_W2 + H2CH  # 1282: per-species weight blob columns
BF16 = ml_dtypes.bfloat16

_graph_cache = {}
_last_run = {}


def _chunks(c):
    """Split capacity c into matmul column tiles of at most MAX_N.

    Smallest tile first so species 0's first x DMA is small and lands
    early.
    """
    sizes = []
    rem = c
    while rem > 0:
        n = min(MAX_N, rem)
        sizes.append(n)
        rem -= n
    sizes.reverse()
    out = []
    n0 = 0
    for n in sizes:
        out.append((n0, n))
        n0 += n
    return out


def _build_graph(caps):
    """One Bass graph, shared by all 8 cores (SPMD)."""
    offs = np.concatenate([[0], np.cumsum(caps)])
    ctot = int(offs[-1])
    f32, bf = mybir.dt.float32, mybir.dt.bfloat16
    Act = mybir.ActivationFunctionType

    nc = bacc.Bacc()
    xt_d = nc.declare_dram_parameter("xt", [128, DCH * ctot], bf, isOutput=False)
    wt_d = nc.declare_dram_parameter("wt", [128, E, WT_COLS], bf, isOutput=False)
    bias_d = nc.declare_dram_parameter("bias", [128, E, 5, 1], f32, isOutput=False)
    out_d = nc.declare_dram_parameter("out", [1, ctot], f32, isOutput=True)

    chunk_lists = [_chunks(int(caps[e])) for e in range(E)]

    with tile.TileContext(nc) as tc:
        with (
            tc.tile_pool(name="wp", bufs=1) as wp,
            tc.tile_pool(name="xp", bufs=1) as xp,
            tc.tile_pool(name="hp", bufs=2) as hp,
            tc.tile_pool(name="op", bufs=1) as op,
            tc.tile_pool(name="pp2", bufs=2, space="PSUM") as pp2,
            tc.tile_pool(name="pp1", bufs=1, space="PSUM") as pp1,
        ):
            wt_s = [
                wp.tile([128, WT_COLS], bf, tag=f"wt_{e}", name=f"wt_{e}")
                for e in range(E)
            ]
            bias_s = wp.tile([128, E, 5, 1], f32, tag="bias")

            # per-chunk x tiles [128, DCH, n]; chunk ci of species e sits at
            # xt_d cols [DCH*(off_e+n0), DCH*(off_e+n0) + DCH*n), laid out
            # [DCH, n] (contiguous per partition).
            x_ts = [
                [
                    xp.tile([128, DCH, n], bf, tag=f"x_{e}_{ci}", name=f"x_{e}_{ci}")
                    for ci, (n0, n) in enumerate(chunk_lists[e])
                ]
                for e in range(E)
            ]

            # --- all input DMAs up front, strict consumption order,
            # alternating between the two HWDGE queues for aggregate BW ---
            dma_seq = [(bias_s[:], bias_d[:])]
            for e in range(E):
                dma_seq.append((wt_s[e][:], wt_d[:, e]))
                for ci, (n0, n) in enumerate(chunk_lists[e]):
                    c0 = DCH * (int(offs[e]) + n0)
                    dma_seq.append((x_ts[e][ci][:], xt_d[:, c0 : c0 + DCH * n]))
            for i, (dst, src) in enumerate(dma_seq):
                eng = nc.sync if i % 2 == 0 else nc.scalar
                eng.dma_start(out=dst, in_=src)

            # --- compute ---
            for e in range(E):
                g0 = int(offs[e])
                en_t = op.tile([1, int(caps[e])], f32, tag=f"en_{e}", name=f"en_{e}")
                for ci, (n0, n) in enumerate(chunk_lists[e]):
                    def rhs_x(d, _t=x_ts[e][ci]):
                        return _t[:, d, :]
                    h1_t = hp.tile([128, H1CH, n], bf, tag="h1", name="h1_t")
                    h2_t = hp.tile([128, H2CH, n], bf, tag="h2", name="h2_t")

                    # layer 1: h1.T = W1.T @ x.T + b1 (no activation)
                    for h in range(H1CH):
                        ps = pp2.tile([128, n], f32, tag=f"ph1_{h}", name="ph1")
                        for d in range(DCH):
                            c0 = WT_W1 + d * (H1CH * 128) + h * 128
                            nc.tensor.matmul(
                                ps[:],
                                lhsT=wt_s[e][:, c0 : c0 + 128],
                                rhs=rhs_x(d),
                                start=(d == 0),
                                stop=(d == DCH - 1),
                            )
                        if h == 0:
                            nc.scalar.activation(
                                h1_t[:, h, :], ps[:], Act.Identity,
                                bias=bias_s[:, e, h, :],
                            )
                        else:
                            nc.vector.tensor_scalar_add(
                                h1_t[:, h, :], ps[:], bias_s[:, e, h, :]
                            )

                    # layer 2: h2.T = relu(Wh.T @ h1.T + bh)
                    for m in range(H2CH):
                        ps = pp1.tile([128, n], f32, tag=f"ph2_{m}", name="ph2")
                        for k in range(H1CH):
                            c0 = WT_WH + k * (H2CH * 128) + m * 128
                            nc.tensor.matmul(
                                ps[:],
                                lhsT=wt_s[e][:, c0 : c0 + 128],
                                rhs=h1_t[:, k, :],
                                start=(k == 0),
                                stop=(k == H1CH - 1),
                            )
                        if m == 0:
                            nc.scalar.activation(
                                h2_t[:, m, :], ps[:], Act.Relu,
                                bias=bias_s[:, e, H1CH + m, :],
                            )
                        else:
                            nc.vector.tensor_scalar(
                                h2_t[:, m, :], ps[:], bias_s[:, e, H1CH + m, :],
                                0.0, mybir.AluOpType.add, mybir.AluOpType.max,
                            )

                    # layer 3: en = W2.T @ h2.T + b2
                    ps_e = pp2.tile([1, n], f32, tag="pen", name="pen")
                    for k in range(H2CH):
                        nc.tensor.matmul(
                            ps_e[:],
                            lhsT=wt_s[e][:, WT_W2 + k : WT_W2 + k + 1],
                            rhs=h2_t[:, k, :],
                            start=(k == 0),
                            stop=(k == H2CH - 1),
                        )
                    nc.scalar.activation(
                        en_t[:, n0 : n0 + n], ps_e[:], Act.Identity,
                        bias=bias_s[0:1, e, 4, :],
                    )
                nc.scalar.dma_start(
                    out=out_d[:, g0 : g0 + int(caps[e])], in_=en_t[:]
                )
    return nc


def _pack_weights(W1, b1, Wh, bh, W2, b2):
    W1 = np.asarray(W1, np.float32)
    b1 = np.asarray(b1, np.float32)
    Wh = np.asarray(Wh, np.float32)
    bh = np.asarray(bh, np.float32)
    W2 = np.asarray(W2, np.float32)
    b2 = np.asarray(b2, np.float32)

    # weight blob [128, E, WT_COLS]: w1 | wh(padded) | w2(padded)
    w1p = W1.reshape(E, DCH, 128, H1CH, 128).transpose(2, 0, 1, 3, 4).reshape(
        128, E, WT_WH
    )
    whpad = np.zeros((E, H1, H2CH * 128), np.float32)
    whpad[:, :, :H2] = Wh
    whp = whpad.reshape(E, H1CH, 128, H2CH, 128).transpose(2, 0, 1, 3, 4).reshape(
        128, E, WT_W2 - WT_WH
    )
    w2pad = np.zeros((E, H2CH * 128), np.float32)
    w2pad[:, :H2] = W2
    w2p = w2pad.reshape(E, H2CH, 128).transpose(2, 0, 1)
    wt = np.concatenate([w1p, whp, w2p], axis=2).astype(BF16)

    # bias blob [128, E, 5, 1]: b1 (2 chunks) | bh (2 chunks) | b2 (row 0)
    bias = np.zeros((128, E, 5, 1), np.float32)
    bias[:, :, 0:H1CH, 0] = b1.reshape(E, H1CH, 128).transpose(2, 0, 1)
    bhpad = np.zeros((E, H2CH * 128), np.float32)
    bhpad[:, :H2] = bh
    bias[:, :, H1CH : H1CH + H2CH, 0] = bhpad.reshape(E, H2CH, 128).transpose(
        2, 0, 1
    )
    bias[0, :, 4, 0] = b2
    return {"wt": np.ascontiguousarray(wt), "bias": bias}


def kernel(representation, species, W1, b1, Wh, bh, W2, b2):
    rep = np.ascontiguousarray(np.asarray(representation, np.float32)).reshape(
        B * A, D
    )
    spec = np.asarray(species).reshape(B * A)

    # --- dispatch: per-species atom lists, split evenly across cores ---
    idx_ce = [[None] * E for _ in range(NCORES)]
    for e in range(E):
        ide = np.nonzero(spec == e)[0]
        for c, part in enumerate(np.array_split(ide, NCORES)):
            idx_ce[c][e] = part
    caps = tuple(
        max(128, int(-(-max(len(idx_ce[c][e]) for c in range(NCORES)) // 64) * 64))
        for e in range(E)
    )
    offs = np.concatenate([[0], np.cumsum(caps)])
    ctot = int(offs[-1])
    chunk_lists = [_chunks(int(caps[e])) for e in range(E)]

    wdict = _pack_weights(W1, b1, Wh, bh, W2, b2)
    rep_bf = rep.astype(BF16)

    in_maps = []
    for c in range(NCORES):
        xt = np.zeros((128, DCH * ctot), BF16)
        for e in range(E):
            ids = idx_ce[c][e]
            for n0, n in chunk_lists[e]:
                sub = ids[n0 : n0 + n]
                r = len(sub)
                if r == 0:
                    continue
                blk = rep_bf[sub].reshape(r, DCH, 128).transpose(2, 1, 0)
                c0 = DCH * (int(offs[e]) + n0)
                for d in range(DCH):
                    xt[:, c0 + d * n : c0 + d * n + r] = blk[:, d, :]
        in_maps.append({"xt": xt, **wdict})

    key = caps
    if key not in _graph_cache:
        nc = _build_graph(caps)
        nc.finalize()
        _graph_cache[key] = nc
    nc = _graph_cache[key]

    res = run_bass_kernel_spmd(nc, in_maps, core_ids=list(range(NCORES)))
    _last_run.update(nc=nc, in_maps=in_maps, caps=caps)

    # --- combine: scatter-add per-atom energies into per-molecule sums ---
    out = np.zeros(B, np.float64)
    for c in range(NCORES):
        en = np.asarray(res.results[c]["out"], np.float64)[0]
        for e in range(E):
            ids = idx_ce[c][e]
            n = len(ids)
            if n:
                out += np.bincount(
                    ids // A,
                    weights=en[int(offs[e]) : int(offs[e]) + n],
                    minlength=B,
                )
    return out.astype(np.float32)


# revision 22
# speedup vs baseline: 1.2999x; 1.0377x over previous
"""Trainium2 Bass kernel for ANI-style per-species MLP (MoE routing).

Reference computation (dense form):
    h  = rep @ W1[e] + b1[e]          (no activation on layer-1 output)
    h  = relu(h @ Wh[e] + bh[e])
    en = h @ W2[e] + b2[e]
    out[b] = sum over atoms of en[b, a, species[b, a]]

Strategy: MoE dispatch on the host — gather atoms by species, split each
species' atom list evenly across the 8 NeuronCores, pad each per-species
group to a static capacity.  Each core runs the same Bass graph (SPMD):
for each species, grouped GEMMs (3-layer MLP) over that species' atom
columns, hidden dim on SBUF partitions so per-partition bias/ReLU
epilogues fuse the PSUM->SBUF copy.  Compute is bf16 with fp32 PSUM
accumulation (rel err vs the fp32 reference ~4e-3).  Per-atom energies
are combined (scatter-add per molecule) on the host.

DMA plan (hard-won constraints): a HWDGE queue sustains ~150-200 GB/s on
~1KB descriptor runs; every dma_start costs ~650ns of issuing-engine
time; and Tile has 8 HW-DMA semaphore lanes, so the (k+8)-th DMA's issue
waits for the k-th DMA's completion.  Hence: few, large, contiguous
transfers (one weight blob per species, one bias blob, one x blob per
species — species 0 split per chunk for a fast start), split across the
two HWDGE queues (sync engine: species 0,2; scalar engine: species 1,3)
in consumption order.  x is stored chunk-major [3, n] per chunk so every
transfer and every matmul rhs slice is contiguous.
"""

import numpy as np
import ml_dtypes

from concourse import bacc
import concourse.mybir as mybir
from concourse import tile
from concourse.bass_utils import run_bass_kernel_spmd

B, A, D, E = 32, 1024, 384, 4
H1, H2 = 256, 192
NCORES = 8
DCH = D // 128          # 3 chunks of the descriptor dim
H1CH = H1 // 128        # 2 chunks of hidden-1
H2CH = 2                # hidden-2 padded 192 -> 256 = 2 chunks
MAX_N = 512             # moving free dim per matmul (one fp32 PSUM bank)
WT_W1, WT_WH, WT_W2 = 0, DCH * H1CH * 128, DCH * H1CH * 128 + H1CH * H2CH * 128
WT_COLS = WT_W2 + H2CH  # 1282: per-species weight blob columns
BF16 = ml_dtypes.bfloat16

_graph_cache = {}
_last_run = {}


def _chunks(c):
    """Split capacity c into matmul column tiles of at most MAX_N.

    Smallest tile first so species 0's first x DMA is small and lands
    early.
    """
    sizes = []
    rem = c
    while rem > 0:
        n = min(MAX_N, rem)
        sizes.append(n)
        rem -= n
    sizes.reverse()
    out = []
    n0 = 0
    for n in sizes:
        out.append((n0, n))
        n0 += n
    return out


def _build_graph(caps):
    """One Bass graph, shared by all 8 cores (SPMD)."""
    offs = np.concatenate([[0], np.cumsum(caps)])
    ctot = int(offs[-1])
    f32, bf = mybir.dt.float32, mybir.dt.bfloat16
    Act = mybir.ActivationFunctionType

    nc = bacc.Bacc()
    xt_d = nc.declare_dram_parameter("xt", [128, DCH * ctot], bf, isOutput=False)
    wt_d = nc.declare_dram_parameter("wt", [128, E, WT_COLS], bf, isOutput=False)
    bias_d = nc.declare_dram_parameter("bias", [128, E, 5, 1], f32, isOutput=False)
    out_d = nc.declare_dram_parameter("out", [1, ctot], f32, isOutput=True)

    chunk_lists = [_chunks(int(caps[e])) for e in range(E)]

    with tile.TileContext(nc) as tc:
        with (
            tc.tile_pool(name="wp", bufs=1) as wp,
            tc.tile_pool(name="xp", bufs=1) as xp,
            tc.tile_pool(name="hp", bufs=2) as hp,
            tc.tile_pool(name="op", bufs=1) as op,
            tc.tile_pool(name="pp2", bufs=2, space="PSUM") as pp2,
            tc.tile_pool(name="pp1", bufs=1, space="PSUM") as pp1,
        ):
            wt_s = [
                wp.tile([128, WT_COLS], bf, tag=f"wt_{e}", name=f"wt_{e}")
                for e in range(E)
            ]
            bias_s = wp.tile([128, E, 5, 1], f32, tag="bias")

            # per-chunk x tiles [128, DCH, n]; chunk ci of species e sits at
            # xt_d cols [DCH*(off_e+n0), DCH*(off_e+n0) + DCH*n), laid out
            # [DCH, n] (contiguous per partition).
            x_ts = [
                [
                    xp.tile([128, DCH, n], bf, tag=f"x_{e}_{ci}", name=f"x_{e}_{ci}")
                    for ci, (n0, n) in enumerate(chunk_lists[e])
                ]
                for e in range(E)
            ]

            # --- all input DMAs up front, strict consumption order,
            # alternating between the two HWDGE queues for aggregate BW ---
            dma_seq = [(bias_s[:], bias_d[:])]
            for e in range(E):
                dma_seq.append((wt_s[e][:], wt_d[:, e]))
                for ci, (n0, n) in enumerate(chunk_lists[e]):
                    c0 = DCH * (int(offs[e]) + n0)
                    dma_seq.append((x_ts[e][ci][:], xt_d[:, c0 : c0 + DCH * n]))
            for i, (dst, src) in enumerate(dma_seq):
                eng = nc.sync if i % 2 == 0 else nc.gpsimd
                eng.dma_start(out=dst, in_=src)

            # --- compute ---
            for e in range(E):
                g0 = int(offs[e])
                en_t = op.tile([1, int(caps[e])], f32, tag=f"en_{e}", name=f"en_{e}")
                for ci, (n0, n) in enumerate(chunk_lists[e]):
                    def rhs_x(d, _t=x_ts[e][ci]):
                        return _t[:, d, :]
                    h1_t = hp.tile([128, H1CH, n], bf, tag="h1", name="h1_t")
                    h2_t = hp.tile([128, H2CH, n], bf, tag="h2", name="h2_t")

                    # layer 1: h1.T = W1.T @ x.T + b1 (no activation)
                    for h in range(H1CH):
                        ps = pp2.tile([128, n], f32, tag=f"ph1_{h}", name="ph1")
                        for d in range(DCH):
                            c0 = WT_W1 + d * (H1CH * 128) + h * 128
                            nc.tensor.matmul(
                                ps[:],
                                lhsT=wt_s[e][:, c0 : c0 + 128],
                                rhs=rhs_x(d),
                                start=(d == 0),
                                stop=(d == DCH - 1),
                            )
                        if h == 0:
                            nc.scalar.activation(
                                h1_t[:, h, :], ps[:], Act.Identity,
                                bias=bias_s[:, e, h, :],
                            )
                        else:
                            nc.vector.tensor_scalar_add(
                                h1_t[:, h, :], ps[:], bias_s[:, e, h, :]
                            )

                    # layer 2: h2.T = relu(Wh.T @ h1.T + bh)
                    for m in range(H2CH):
                        ps = pp1.tile([128, n], f32, tag=f"ph2_{m}", name="ph2")
                        for k in range(H1CH):
                            c0 = WT_WH + k * (H2CH * 128) + m * 128
                            nc.tensor.matmul(
                                ps[:],
                                lhsT=wt_s[e][:, c0 : c0 + 128],
                                rhs=h1_t[:, k, :],
                                start=(k == 0),
                                stop=(k == H1CH - 1),
                            )
                        if m == 0:
                            nc.scalar.activation(
                                h2_t[:, m, :], ps[:], Act.Relu,
                                bias=bias_s[:, e, H1CH + m, :],
                            )
                        else:
                            nc.vector.tensor_scalar(
                                h2_t[:, m, :], ps[:], bias_s[:, e, H1CH + m, :],
                                0.0, mybir.AluOpType.add, mybir.AluOpType.max,
                            )

                    # layer 3: en = W2.T @ h2.T + b2
                    ps_e = pp2.tile([1, n], f32, tag="pen", name="pen")
                    for k in range(H2CH):
                        nc.tensor.matmul(
                            ps_e[:],
                            lhsT=wt_s[e][:, WT_W2 + k : WT_W2 + k + 1],
                            rhs=h2_t[:, k, :],
                            start=(k == 0),
                            stop=(k == H2CH - 1),
                        )
                    nc.scalar.activation(
                        en_t[:, n0 : n0 + n], ps_e[:], Act.Identity,
                        bias=bias_s[0:1, e, 4, :],
                    )
                nc.sync.dma_start(
                    out=out_d[:, g0 : g0 + int(caps[e])], in_=en_t[:]
                )
    return nc


def _pack_weights(W1, b1, Wh, bh, W2, b2):
    W1 = np.asarray(W1, np.float32)
    b1 = np.asarray(b1, np.float32)
    Wh = np.asarray(Wh, np.float32)
    bh = np.asarray(bh, np.float32)
    W2 = np.asarray(W2, np.float32)
    b2 = np.asarray(b2, np.float32)

    # weight blob [128, E, WT_COLS]: w1 | wh(padded) | w2(padded)
    w1p = W1.reshape(E, DCH, 128, H1CH, 128).transpose(2, 0, 1, 3, 4).reshape(
        128, E, WT_WH
    )
    whpad = np.zeros((E, H1, H2CH * 128), np.float32)
    whpad[:, :, :H2] = Wh
    whp = whpad.reshape(E, H1CH, 128, H2CH, 128).transpose(2, 0, 1, 3, 4).reshape(
        128, E, WT_W2 - WT_WH
    )
    w2pad = np.zeros((E, H2CH * 128), np.float32)
    w2pad[:, :H2] = W2
    w2p = w2pad.reshape(E, H2CH, 128).transpose(2, 0, 1)
    wt = np.concatenate([w1p, whp, w2p], axis=2).astype(BF16)

    # bias blob [128, E, 5, 1]: b1 (2 chunks) | bh (2 chunks) | b2 (row 0)
    bias = np.zeros((128, E, 5, 1), np.float32)
    bias[:, :, 0:H1CH, 0] = b1.reshape(E, H1CH, 128).transpose(2, 0, 1)
    bhpad = np.zeros((E, H2CH * 128), np.float32)
    bhpad[:, :H2] = bh
    bias[:, :, H1CH : H1CH + H2CH, 0] = bhpad.reshape(E, H2CH, 128).transpose(
        2, 0, 1
    )
    bias[0, :, 4, 0] = b2
    return {"wt": np.ascontiguousarray(wt), "bias": bias}


def kernel(representation, species, W1, b1, Wh, bh, W2, b2):
    rep = np.ascontiguousarray(np.asarray(representation, np.float32)).reshape(
        B * A, D
    )
    spec = np.asarray(species).reshape(B * A)

    # --- dispatch: per-species atom lists, split evenly across cores ---
    idx_ce = [[None] * E for _ in range(NCORES)]
    for e in range(E):
        ide = np.nonzero(spec == e)[0]
        for c, part in enumerate(np.array_split(ide, NCORES)):
            idx_ce[c][e] = part
    caps = tuple(
        max(128, int(-(-max(len(idx_ce[c][e]) for c in range(NCORES)) // 64) * 64))
        for e in range(E)
    )
    offs = np.concatenate([[0], np.cumsum(caps)])
    ctot = int(offs[-1])
    chunk_lists = [_chunks(int(caps[e])) for e in range(E)]

    wdict = _pack_weights(W1, b1, Wh, bh, W2, b2)
    rep_bf = rep.astype(BF16)

    in_maps = []
    for c in range(NCORES):
        xt = np.zeros((128, DCH * ctot), BF16)
        for e in range(E):
            ids = idx_ce[c][e]
            for n0, n in chunk_lists[e]:
                sub = ids[n0 : n0 + n]
                r = len(sub)
                if r == 0:
                    continue
                blk = rep_bf[sub].reshape(r, DCH, 128).transpose(2, 1, 0)
                c0 = DCH * (int(offs[e]) + n0)
                for d in range(DCH):
                    xt[:, c0 + d * n : c0 + d * n + r] = blk[:, d, :]
        in_maps.append({"xt": xt, **wdict})

    key = caps
    if key not in _graph_cache:
        nc = _build_graph(caps)
        nc.finalize()
        _graph_cache[key] = nc
    nc = _graph_cache[key]

    res = run_bass_kernel_spmd(nc, in_maps, core_ids=list(range(NCORES)))
    _last_run.update(nc=nc, in_maps=in_maps, caps=caps)

    # --- combine: scatter-add per-atom energies into per-molecule sums ---
    out = np.zeros(B, np.float64)
    for c in range(NCORES):
        en = np.asarray(res.results[c]["out"], np.float64)[0]
        for e in range(E):
            ids = idx_ce[c][e]
            n = len(ids)
            if n:
                out += np.bincount(
                    ids // A,
                    weights=en[int(offs[e]) : int(offs[e]) + n],
                    minlength=B,
                )
    return out.astype(np.float32)
